# revision 11
# baseline (speedup 1.0000x reference)
"""Adaptive Kalman filter NN kernel for 8 TRN2 NeuronCores (Bass/Tile).

Structure exploited (mirrors the reference exactly, for any inputs of the
fixed shapes):
  - The scan carry returns (state, P_upd) where `state` is only reassigned
    on resets (state <- observation[t]); the filtered update never feeds
    back. So state_t is piecewise constant across reset segments.
  - The covariance recursion P/K is (d,d), batch-independent, and depends
    only on A,H,Q,R and the reset schedule -> computed on host (tiny).
  - Device work is the per-step batch GEMMs, time-sharded over 8 cores:
        paB_t  = pa_t @ B^T
        paBH_t = pa_t @ (H B)^T
        errs_t = ob'_t - paBH_t          (ob' = ob - state_seg A^T H^T, host)
        upds'_t = paB_t + errs_t @ K_t^T (upds = upds' + state_seg A^T, host)
    All tensors are kept feature-major on chip (d on partitions), two time
    steps packed per 128-partition tile; B/BH weights are block-diagonal
    over a quad of steps so each matmul runs the full 128x128 stationary
    array with N=512 moving (two quads); per-pair K matmuls accumulate on
    top of the paB PSUM tile at N=256.

Matmul operands are float32r (TF32): 1 cycle/row at N>=256 vs 4 for fp32.
The host pre-rounds pa/K/weights to tf32; errs is rounded once by the DVE
subtract writing an f32r tile (errs output is tf32-rounded, ~1e-4 rel).

K is shipped compact (per-step 64x64, 2MB/core) and expanded on device
into two persistent zero-initialized block-diagonal tiles (the zeros are
memset once and never rewritten).

Per-block pair permutation: a B-matmul over quads (2q, 2q+1) with the
"even" weight produces pairs (4g, 4g+2) in one PSUM tile, so pairs are
stored block-locally in order [0,2,1,3,4,6,5,7]; the host packs ob2/k2 in
that order and unpermutes the outputs.
"""

import numpy as np

import concourse.bass as bass
import concourse.mybir as mybir
from concourse import bacc
from concourse.tile import TileContext
from concourse.bass_utils import run_bass_kernel_spmd

EPS = 1e-6
T, BATCH, D, A_DIM = 1024, 256, 64, 32
N_CORES = 8
T_LOC = T // N_CORES          # 128 steps per core
PAIRS = T_LOC // 2            # 64 pairs per core
QUADS = T_LOC // 4            # 32 quads per core
BLK_PAIRS = 8                 # pairs per DMA block
N_BLK = PAIRS // BLK_PAIRS    # 8 blocks per core
PERM = [0, 2, 1, 3, 4, 6, 5, 7]   # block-local pair storage order

_NC_CACHE = None

# exec time of last run (ns) when BASS_TRACE=1 and the ntff hook is live
LAST_EXEC_NS = None


def _build_nc():
    nc = bacc.Bacc()
    f32 = mybir.dt.float32
    f32r = mybir.dt.float32r

    pa4 = nc.declare_dram_parameter("pa4", [QUADS, 128, BATCH], f32r, isOutput=False)
    ob2 = nc.declare_dram_parameter("ob2", [PAIRS, 128, BATCH], f32, isOutput=False)
    # compact K, per-block packed: [blk][partition][pair*64 + col]
    k2 = nc.declare_dram_parameter("k2", [N_BLK, 128, BLK_PAIRS * 64], f32r,
                                   isOutput=False)
    # stacked [wbb_a, wbb_b, wbh_a, wbh_b]
    wts = nc.declare_dram_parameter("wts", [4, 128, 128], f32r, isOutput=False)
    kzero = nc.declare_dram_parameter("kzero", [128, BLK_PAIRS, 128], f32r,
                                      isOutput=False)
    upds2 = nc.declare_dram_parameter("upds2", [PAIRS, 128, BATCH], f32, isOutput=True)
    errs2 = nc.declare_dram_parameter("errs2", [PAIRS, 128, BATCH], f32r, isOutput=True)

    mm = nc.tensor.matmul

    with TileContext(nc) as tc:
        with (
            tc.tile_pool(name="const", bufs=1) as cpool,
            tc.tile_pool(name="sbuf", bufs=3) as pool,
            tc.tile_pool(name="psum0", bufs=2, space="PSUM") as p0pool,
            tc.tile_pool(name="psum1", bufs=2, space="PSUM") as p1pool,
        ):
            wts_sb = cpool.tile([128, 4, 128], f32r, name="wts_sb")
            nc.sync.dma_start(out=wts_sb[:], in_=wts.rearrange("w k n -> k w n"))
            wbb = [wts_sb[:, 0], wts_sb[:, 1]]
            wbh = [wts_sb[:, 2], wts_sb[:, 3]]

            # persistent block-diagonal K tiles; zeros written once
            k_bd = [cpool.tile([128, BLK_PAIRS, 128], f32r, name="k_bd0"),
                    cpool.tile([128, BLK_PAIRS, 128], f32r, name="k_bd1")]
            nc.sync.dma_start(out=k_bd[0][:], in_=kzero[:])
            nc.sync.dma_start(out=k_bd[1][:], in_=kzero[:])

            for blk in range(N_BLK):
                sp = blk * BLK_PAIRS
                sq = blk * (BLK_PAIRS // 2)

                pa_sb = pool.tile([128, BLK_PAIRS // 2, BATCH], f32r, tag="pa")
                nc.sync.dma_start(
                    out=pa_sb[:],
                    in_=pa4[sq : sq + BLK_PAIRS // 2].rearrange("q k n -> k q n"),
                )
                ob_sb = pool.tile([128, BLK_PAIRS, BATCH], f32, tag="ob")
                nc.sync.dma_start(
                    out=ob_sb[:],
                    in_=ob2[sp : sp + BLK_PAIRS].rearrange("p k n -> k p n"),
                )
                k_sb = pool.tile([128, BLK_PAIRS, 64], f32r, tag="k")
                nc.sync.dma_start(
                    out=k_sb[:],
                    in_=k2[blk].rearrange("k (p n) -> k p n", n=64),
                )
                kb = k_bd[blk % 2]
                nc.vector.tensor_copy(kb[0:64, :, 0:64], k_sb[0:64])
                nc.vector.tensor_copy(kb[64:128, :, 64:128], k_sb[64:128])

                errs_sb = pool.tile([128, BLK_PAIRS, BATCH], f32r, tag="errs")
                upds_sb = pool.tile([128, BLK_PAIRS, BATCH], f32, tag="upds")

                for g in range(BLK_PAIRS // 4):   # group of 2 quads / 4 pairs
                    q0 = 2 * g
                    s0 = 4 * g                    # first storage slot of group
                    pa_mv = pa_sb[:, q0 : q0 + 2]  # (128, 512) moving

                    p1e = p1pool.tile([128, 2 * BATCH], f32, tag="p1e", name="p1e")
                    p1o = p1pool.tile([128, 2 * BATCH], f32, tag="p1o", name="p1o")
                    mm(p1e[:], wbh[0], pa_mv, start=True, stop=True)
                    mm(p1o[:], wbh[1], pa_mv, start=True, stop=True)
                    nc.vector.tensor_sub(
                        errs_sb[:, s0 : s0 + 2], ob_sb[:, s0 : s0 + 2], p1e[:]
                    )
                    nc.vector.tensor_sub(
                        errs_sb[:, s0 + 2 : s0 + 4], ob_sb[:, s0 + 2 : s0 + 4], p1o[:]
                    )

                    p0e = p0pool.tile([128, 2 * BATCH], f32, tag="p0e", name="p0e")
                    p0o = p0pool.tile([128, 2 * BATCH], f32, tag="p0o", name="p0o")
                    mm(p0e[:], wbb[0], pa_mv, start=True, stop=False)
                    mm(p0o[:], wbb[1], pa_mv, start=True, stop=False)
                    mm(p0e[:, 0:BATCH], kb[:, s0], errs_sb[:, s0],
                       start=False, stop=False)
                    mm(p0e[:, BATCH : 2 * BATCH], kb[:, s0 + 1], errs_sb[:, s0 + 1],
                       start=False, stop=True)
                    mm(p0o[:, 0:BATCH], kb[:, s0 + 2], errs_sb[:, s0 + 2],
                       start=False, stop=False)
                    mm(p0o[:, BATCH : 2 * BATCH], kb[:, s0 + 3], errs_sb[:, s0 + 3],
                       start=False, stop=True)
                    nc.any.tensor_copy(upds_sb[:, s0 : s0 + 2], p0e[:])
                    nc.any.tensor_copy(upds_sb[:, s0 + 2 : s0 + 4], p0o[:])

                for g in range(BLK_PAIRS // 4):
                    s0 = 4 * g
                    nc.gpsimd.dma_start(
                        out=errs2[sp + s0 : sp + s0 + 4].rearrange("p k n -> k p n"),
                        in_=errs_sb[:, s0 : s0 + 4],
                    )
                    nc.gpsimd.dma_start(
                        out=upds2[sp + s0 : sp + s0 + 4].rearrange("p k n -> k p n"),
                        in_=upds_sb[:, s0 : s0 + 4],
                    )
    return nc


def _get_nc():
    global _NC_CACHE
    if _NC_CACHE is None:
        nc = _build_nc()
        nc.finalize()
        _NC_CACHE = nc
    return _NC_CACHE


def _tf32_round(x):
    u = np.ascontiguousarray(x, dtype=np.float32).view(np.uint32)
    lsb = (u >> np.uint32(13)) & np.uint32(1)
    u = (u + np.uint32(0x0FFF) + lsb) & np.uint32(0xFFFFE000)
    return u.view(np.float32)


def _kalman_gains(resets, A, B, H, L_Q, L_R):
    """Host (d,d) covariance recursion; returns K_t for all T steps (f32)."""
    I = np.eye(D, dtype=np.float64)
    A64, H64 = A.astype(np.float64), H.astype(np.float64)
    Q = (L_Q @ L_Q.T).astype(np.float64)
    R = (L_R @ L_R.T).astype(np.float64)
    Ks = np.empty((T, D, D), dtype=np.float32)
    P = I.copy()
    for t in range(T):
        if resets[t]:
            P = I.copy()
        P_pred = A64 @ (P @ A64.T) + Q
        HP = P_pred @ H64.T
        S = H64 @ HP + R + EPS * I
        K = HP @ np.linalg.inv(S)
        Ks[t] = K.astype(np.float32)
        left = I - K @ H64
        P = left @ P_pred @ left.T + K @ R @ K.T
    return Ks


def kernel(state_estimate, previous_action, current_action, observation, is_init,
           A, B, H, L_Q, L_R):
    global LAST_EXEC_NS
    se = np.asarray(state_estimate, dtype=np.float32)
    pa = np.asarray(previous_action, dtype=np.float32)
    ca = np.asarray(current_action)
    ob = np.asarray(observation, dtype=np.float32)
    ii = np.asarray(is_init)
    A = np.asarray(A, dtype=np.float32)
    B = np.asarray(B, dtype=np.float32)
    H = np.asarray(H, dtype=np.float32)
    L_Q = np.asarray(L_Q, dtype=np.float32)
    L_R = np.asarray(L_R, dtype=np.float32)

    resets = np.any(ii, axis=1)

    Ks = _kalman_gains(resets, A, B, H, L_Q, L_R)

    # --- segments of piecewise-constant carry state ---
    seg_starts = [0] + [int(t) for t in np.nonzero(resets)[0]]
    segs = []  # (t0, t1, sA) with sA = state_seg @ A.T
    for i, t0 in enumerate(seg_starts):
        t1 = seg_starts[i + 1] if i + 1 < len(seg_starts) else T
        if t1 <= t0:
            continue
        st = se[0] if t0 == 0 and not resets[0] else ob[t0]
        segs.append((t0, t1, (st @ A.T).astype(np.float32)))

    # --- host pre-adjust ob' = ob - sA @ H.T ---
    obp = ob.copy()
    for (t0, t1, sA) in segs:
        obp[t0:t1] -= (sA @ H.T)[None, :, :]

    # --- device-layout packing (feature-major, 2 steps per 128 partitions) ---
    obT = np.ascontiguousarray(obp.transpose(0, 2, 1))      # (T, 64, 256)
    ob2_all = obT.reshape(T // 2, 128, BATCH)
    paT = _tf32_round(np.ascontiguousarray(pa.transpose(0, 2, 1)))  # (T, 32, 256)
    pa4_all = paT.reshape(T // 4, 128, BATCH)

    k2_all = np.empty((T // 2, 128, 64), dtype=np.float32)
    KsT = _tf32_round(Ks.transpose(0, 2, 1))                # K_t^T
    k2_all[:, 0:64] = KsT[0::2]
    k2_all[:, 64:128] = KsT[1::2]

    BT = np.ascontiguousarray(B.T)                          # (32, 64)
    HBT = np.ascontiguousarray((H @ B).T)                   # (32, 64)
    wts = np.zeros((4, 128, 128), dtype=np.float32)
    wts[0, 0:32, 0:64] = BT      # wbb_a
    wts[0, 32:64, 64:128] = BT
    wts[1, 64:96, 0:64] = BT     # wbb_b
    wts[1, 96:128, 64:128] = BT
    wts[2, 0:32, 0:64] = HBT     # wbh_a
    wts[2, 32:64, 64:128] = HBT
    wts[3, 64:96, 0:64] = HBT    # wbh_b
    wts[3, 96:128, 64:128] = HBT
    wts = _tf32_round(wts)

    # block-local pair permutation (storage order on device)
    order = np.concatenate(
        [b * BLK_PAIRS + np.array(PERM) for b in range(T // 2 // BLK_PAIRS)]
    )
    inv_order = np.argsort(order)

    ob2_perm = ob2_all[order]
    # pack K per block: (n_blocks, 128, 8*64), block-contiguous per partition
    k2_perm = (k2_all[order]
               .reshape(-1, BLK_PAIRS, 128, 64)
               .transpose(0, 2, 1, 3)
               .reshape(-1, 128, BLK_PAIRS * 64))

    kzero_arr = np.zeros((128, BLK_PAIRS, 128), dtype=np.float32)
    in_maps = []
    for c in range(N_CORES):
        in_maps.append({
            "pa4": np.ascontiguousarray(pa4_all[c * QUADS:(c + 1) * QUADS]),
            "ob2": np.ascontiguousarray(ob2_perm[c * PAIRS:(c + 1) * PAIRS]),
            "k2": np.ascontiguousarray(k2_perm[c * N_BLK:(c + 1) * N_BLK]),
            "wts": wts,
            "kzero": kzero_arr,
        })

    nc = _get_nc()
    res = run_bass_kernel_spmd(nc, in_maps, core_ids=list(range(N_CORES)))
    LAST_EXEC_NS = res.exec_time_ns

    upds2_full = np.concatenate([np.asarray(res.results[c]["upds2"]) for c in range(N_CORES)], axis=0)
    errs2_full = np.concatenate([np.asarray(res.results[c]["errs2"]) for c in range(N_CORES)], axis=0)
    upds2_full = upds2_full[inv_order]
    errs2_full = errs2_full[inv_order]

    upds = np.ascontiguousarray(
        upds2_full.reshape(T, D, BATCH).transpose(0, 2, 1))
    errs = np.ascontiguousarray(
        errs2_full.reshape(T, D, BATCH).transpose(0, 2, 1))

    # --- host post-adjust upds += sA per segment ---
    for (t0, t1, sA) in segs:
        upds[t0:t1] += sA[None, :, :]

    return upds, ca, errs


# revision 12
# speedup vs baseline: 1.0859x; 1.0859x over previous
"""Adaptive Kalman filter NN kernel for 8 TRN2 NeuronCores (Bass/Tile).

Structure exploited (mirrors the reference exactly, for any inputs of the
fixed shapes):
  - The scan carry returns (state, P_upd) where `state` is only reassigned
    on resets (state <- observation[t]); the filtered update never feeds
    back. So state_t is piecewise constant across reset segments.
  - The covariance recursion P/K is (d,d), batch-independent, and depends
    only on A,H,Q,R and the reset schedule -> computed on host (tiny).
  - Device work is the per-step batch GEMMs, time-sharded over 8 cores:
        paB_t  = pa_t @ B^T
        paBH_t = pa_t @ (H B)^T
        errs_t = ob'_t - paBH_t          (ob' = ob - state_seg A^T H^T, host)
        upds'_t = paB_t + errs_t @ K_t^T (upds = upds' + state_seg A^T, host)
    All tensors are kept feature-major on chip (d on partitions), two time
    steps packed per 128-partition tile; B/BH weights are block-diagonal
    over a quad of steps so each matmul runs the full 128x128 stationary
    array with N=512 moving (two quads); per-pair K matmuls accumulate on
    top of the paB PSUM tile at N=256.

Matmul operands are float32r (TF32): 1 cycle/row at N>=256 vs 4 for fp32.
The host pre-rounds pa/K/weights to tf32; errs is rounded once by the DVE
subtract writing an f32r tile (errs output is tf32-rounded, ~1e-4 rel).

K is shipped compact (per-step 64x64, 2MB/core) and expanded on device
into two persistent zero-initialized block-diagonal tiles (the zeros are
memset once and never rewritten).

Per-block pair permutation: a B-matmul over quads (2q, 2q+1) with the
"even" weight produces pairs (4g, 4g+2) in one PSUM tile, so pairs are
stored block-locally in order [0,2,1,3,4,6,5,7]; the host packs ob2/k2 in
that order and unpermutes the outputs.
"""

import numpy as np

import concourse.bass as bass
import concourse.mybir as mybir
from concourse import bacc
from concourse.tile import TileContext
from concourse.bass_utils import run_bass_kernel_spmd

EPS = 1e-6
T, BATCH, D, A_DIM = 1024, 256, 64, 32
N_CORES = 8
T_LOC = T // N_CORES          # 128 steps per core
PAIRS = T_LOC // 2            # 64 pairs per core
QUADS = T_LOC // 4            # 32 quads per core
BLK_PAIRS = 8                 # pairs per DMA block
N_BLK = PAIRS // BLK_PAIRS    # 8 blocks per core
PERM = [0, 2, 1, 3, 4, 6, 5, 7]   # block-local pair storage order

_NC_CACHE = None

# exec time of last run (ns) when BASS_TRACE=1 and the ntff hook is live
LAST_EXEC_NS = None


def _build_nc():
    nc = bacc.Bacc()
    f32 = mybir.dt.float32
    f32r = mybir.dt.float32r

    pa4 = nc.declare_dram_parameter("pa4", [QUADS, 128, BATCH], f32r, isOutput=False)
    ob2 = nc.declare_dram_parameter("ob2", [PAIRS, 128, BATCH], f32, isOutput=False)
    # compact K, per-block packed: [blk][partition][pair*64 + col]
    k2 = nc.declare_dram_parameter("k2", [N_BLK, 128, BLK_PAIRS * 64], f32r,
                                   isOutput=False)
    # stacked [wbb_a, wbb_b, wbh_a, wbh_b]
    wts = nc.declare_dram_parameter("wts", [4, 128, 128], f32r, isOutput=False)
    kzero = nc.declare_dram_parameter("kzero", [128, BLK_PAIRS, 128], f32r,
                                      isOutput=False)
    upds2 = nc.declare_dram_parameter("upds2", [PAIRS, 128, BATCH], f32, isOutput=True)
    errs2 = nc.declare_dram_parameter("errs2", [PAIRS, 128, BATCH], f32r, isOutput=True)

    mm = nc.tensor.matmul

    with TileContext(nc) as tc:
        with (
            tc.tile_pool(name="const", bufs=1) as cpool,
            tc.tile_pool(name="sbuf", bufs=4) as pool,
            tc.tile_pool(name="psum0", bufs=2, space="PSUM") as p0pool,
            tc.tile_pool(name="psum1", bufs=2, space="PSUM") as p1pool,
        ):
            wts_sb = cpool.tile([128, 4, 128], f32r, name="wts_sb")
            nc.sync.dma_start(out=wts_sb[:], in_=wts.rearrange("w k n -> k w n"))
            wbb = [wts_sb[:, 0], wts_sb[:, 1]]
            wbh = [wts_sb[:, 2], wts_sb[:, 3]]

            # persistent block-diagonal K tiles; zeros written once
            k_bd = [cpool.tile([128, BLK_PAIRS, 128], f32r, name="k_bd0"),
                    cpool.tile([128, BLK_PAIRS, 128], f32r, name="k_bd1")]

            for blk in range(N_BLK):
                sp = blk * BLK_PAIRS
                sq = blk * (BLK_PAIRS // 2)

                pa_sb = pool.tile([128, BLK_PAIRS // 2, BATCH], f32r, tag="pa")
                ob_sb = pool.tile([128, BLK_PAIRS, BATCH], f32, tag="ob")
                for h in range(2):
                    nc.sync.dma_start(
                        out=pa_sb[:, 2 * h : 2 * h + 2],
                        in_=pa4[sq + 2 * h : sq + 2 * h + 2].rearrange("q k n -> k q n"),
                    )
                    nc.sync.dma_start(
                        out=ob_sb[:, 4 * h : 4 * h + 4],
                        in_=ob2[sp + 4 * h : sp + 4 * h + 4].rearrange("p k n -> k p n"),
                    )
                k_sb = pool.tile([128, BLK_PAIRS, 64], f32r, tag="k")
                nc.sync.dma_start(
                    out=k_sb[:],
                    in_=k2[blk].rearrange("k (p n) -> k p n", n=64),
                )
                if blk == 0:
                    nc.sync.dma_start(out=k_bd[0][:], in_=kzero[:])
                    nc.sync.dma_start(out=k_bd[1][:], in_=kzero[:])
                kb = k_bd[blk % 2]
                nc.vector.tensor_copy(kb[0:64, :, 0:64], k_sb[0:64])
                nc.vector.tensor_copy(kb[64:128, :, 64:128], k_sb[64:128])

                errs_sb = pool.tile([128, BLK_PAIRS, BATCH], f32r, tag="errs")
                upds_sb = pool.tile([128, BLK_PAIRS, BATCH], f32, tag="upds")

                for g in range(BLK_PAIRS // 4):   # group of 2 quads / 4 pairs
                    q0 = 2 * g
                    s0 = 4 * g                    # first storage slot of group
                    pa_mv = pa_sb[:, q0 : q0 + 2]  # (128, 512) moving

                    p1e = p1pool.tile([128, 2 * BATCH], f32, tag="p1e", name="p1e")
                    p1o = p1pool.tile([128, 2 * BATCH], f32, tag="p1o", name="p1o")
                    mm(p1e[:], wbh[0], pa_mv, start=True, stop=True)
                    mm(p1o[:], wbh[1], pa_mv, start=True, stop=True)
                    nc.vector.tensor_sub(
                        errs_sb[:, s0 : s0 + 2], ob_sb[:, s0 : s0 + 2], p1e[:]
                    )
                    nc.vector.tensor_sub(
                        errs_sb[:, s0 + 2 : s0 + 4], ob_sb[:, s0 + 2 : s0 + 4], p1o[:]
                    )

                    p0e = p0pool.tile([128, 2 * BATCH], f32, tag="p0e", name="p0e")
                    p0o = p0pool.tile([128, 2 * BATCH], f32, tag="p0o", name="p0o")
                    mm(p0e[:], wbb[0], pa_mv, start=True, stop=False)
                    mm(p0o[:], wbb[1], pa_mv, start=True, stop=False)
                    mm(p0e[:, 0:BATCH], kb[:, s0], errs_sb[:, s0],
                       start=False, stop=False)
                    mm(p0e[:, BATCH : 2 * BATCH], kb[:, s0 + 1], errs_sb[:, s0 + 1],
                       start=False, stop=True)
                    mm(p0o[:, 0:BATCH], kb[:, s0 + 2], errs_sb[:, s0 + 2],
                       start=False, stop=False)
                    mm(p0o[:, BATCH : 2 * BATCH], kb[:, s0 + 3], errs_sb[:, s0 + 3],
                       start=False, stop=True)
                    nc.any.tensor_copy(upds_sb[:, s0 : s0 + 2], p0e[:])
                    nc.any.tensor_copy(upds_sb[:, s0 + 2 : s0 + 4], p0o[:])

                for g in range(BLK_PAIRS // 4):
                    s0 = 4 * g
                    nc.gpsimd.dma_start(
                        out=errs2[sp + s0 : sp + s0 + 4].rearrange("p k n -> k p n"),
                        in_=errs_sb[:, s0 : s0 + 4],
                    )
                    nc.gpsimd.dma_start(
                        out=upds2[sp + s0 : sp + s0 + 4].rearrange("p k n -> k p n"),
                        in_=upds_sb[:, s0 : s0 + 4],
                    )
    return nc


def _get_nc():
    global _NC_CACHE
    if _NC_CACHE is None:
        nc = _build_nc()
        nc.finalize()
        _NC_CACHE = nc
    return _NC_CACHE


def _tf32_round(x):
    u = np.ascontiguousarray(x, dtype=np.float32).view(np.uint32)
    lsb = (u >> np.uint32(13)) & np.uint32(1)
    u = (u + np.uint32(0x0FFF) + lsb) & np.uint32(0xFFFFE000)
    return u.view(np.float32)


def _kalman_gains(resets, A, B, H, L_Q, L_R):
    """Host (d,d) covariance recursion; returns K_t for all T steps (f32)."""
    I = np.eye(D, dtype=np.float64)
    A64, H64 = A.astype(np.float64), H.astype(np.float64)
    Q = (L_Q @ L_Q.T).astype(np.float64)
    R = (L_R @ L_R.T).astype(np.float64)
    Ks = np.empty((T, D, D), dtype=np.float32)
    P = I.copy()
    for t in range(T):
        if resets[t]:
            P = I.copy()
        P_pred = A64 @ (P @ A64.T) + Q
        HP = P_pred @ H64.T
        S = H64 @ HP + R + EPS * I
        K = HP @ np.linalg.inv(S)
        Ks[t] = K.astype(np.float32)
        left = I - K @ H64
        P = left @ P_pred @ left.T + K @ R @ K.T
    return Ks


def kernel(state_estimate, previous_action, current_action, observation, is_init,
           A, B, H, L_Q, L_R):
    global LAST_EXEC_NS
    se = np.asarray(state_estimate, dtype=np.float32)
    pa = np.asarray(previous_action, dtype=np.float32)
    ca = np.asarray(current_action)
    ob = np.asarray(observation, dtype=np.float32)
    ii = np.asarray(is_init)
    A = np.asarray(A, dtype=np.float32)
    B = np.asarray(B, dtype=np.float32)
    H = np.asarray(H, dtype=np.float32)
    L_Q = np.asarray(L_Q, dtype=np.float32)
    L_R = np.asarray(L_R, dtype=np.float32)

    resets = np.any(ii, axis=1)

    Ks = _kalman_gains(resets, A, B, H, L_Q, L_R)

    # --- segments of piecewise-constant carry state ---
    seg_starts = [0] + [int(t) for t in np.nonzero(resets)[0]]
    segs = []  # (t0, t1, sA) with sA = state_seg @ A.T
    for i, t0 in enumerate(seg_starts):
        t1 = seg_starts[i + 1] if i + 1 < len(seg_starts) else T
        if t1 <= t0:
            continue
        st = se[0] if t0 == 0 and not resets[0] else ob[t0]
        segs.append((t0, t1, (st @ A.T).astype(np.float32)))

    # --- host pre-adjust ob' = ob - sA @ H.T ---
    obp = ob.copy()
    for (t0, t1, sA) in segs:
        obp[t0:t1] -= (sA @ H.T)[None, :, :]

    # --- device-layout packing (feature-major, 2 steps per 128 partitions) ---
    obT = np.ascontiguousarray(obp.transpose(0, 2, 1))      # (T, 64, 256)
    ob2_all = obT.reshape(T // 2, 128, BATCH)
    paT = _tf32_round(np.ascontiguousarray(pa.transpose(0, 2, 1)))  # (T, 32, 256)
    pa4_all = paT.reshape(T // 4, 128, BATCH)

    k2_all = np.empty((T // 2, 128, 64), dtype=np.float32)
    KsT = _tf32_round(Ks.transpose(0, 2, 1))                # K_t^T
    k2_all[:, 0:64] = KsT[0::2]
    k2_all[:, 64:128] = KsT[1::2]

    BT = np.ascontiguousarray(B.T)                          # (32, 64)
    HBT = np.ascontiguousarray((H @ B).T)                   # (32, 64)
    wts = np.zeros((4, 128, 128), dtype=np.float32)
    wts[0, 0:32, 0:64] = BT      # wbb_a
    wts[0, 32:64, 64:128] = BT
    wts[1, 64:96, 0:64] = BT     # wbb_b
    wts[1, 96:128, 64:128] = BT
    wts[2, 0:32, 0:64] = HBT     # wbh_a
    wts[2, 32:64, 64:128] = HBT
    wts[3, 64:96, 0:64] = HBT    # wbh_b
    wts[3, 96:128, 64:128] = HBT
    wts = _tf32_round(wts)

    # block-local pair permutation (storage order on device)
    order = np.concatenate(
        [b * BLK_PAIRS + np.array(PERM) for b in range(T // 2 // BLK_PAIRS)]
    )
    inv_order = np.argsort(order)

    ob2_perm = ob2_all[order]
    # pack K per block: (n_blocks, 128, 8*64), block-contiguous per partition
    k2_perm = (k2_all[order]
               .reshape(-1, BLK_PAIRS, 128, 64)
               .transpose(0, 2, 1, 3)
               .reshape(-1, 128, BLK_PAIRS * 64))

    kzero_arr = np.zeros((128, BLK_PAIRS, 128), dtype=np.float32)
    in_maps = []
    for c in range(N_CORES):
        in_maps.append({
            "pa4": np.ascontiguousarray(pa4_all[c * QUADS:(c + 1) * QUADS]),
            "ob2": np.ascontiguousarray(ob2_perm[c * PAIRS:(c + 1) * PAIRS]),
            "k2": np.ascontiguousarray(k2_perm[c * N_BLK:(c + 1) * N_BLK]),
            "wts": wts,
            "kzero": kzero_arr,
        })

    nc = _get_nc()
    res = run_bass_kernel_spmd(nc, in_maps, core_ids=list(range(N_CORES)))
    LAST_EXEC_NS = res.exec_time_ns

    upds2_full = np.concatenate([np.asarray(res.results[c]["upds2"]) for c in range(N_CORES)], axis=0)
    errs2_full = np.concatenate([np.asarray(res.results[c]["errs2"]) for c in range(N_CORES)], axis=0)
    upds2_full = upds2_full[inv_order]
    errs2_full = errs2_full[inv_order]

    upds = np.ascontiguousarray(
        upds2_full.reshape(T, D, BATCH).transpose(0, 2, 1))
    errs = np.ascontiguousarray(
        errs2_full.reshape(T, D, BATCH).transpose(0, 2, 1))

    # --- host post-adjust upds += sA per segment ---
    for (t0, t1, sA) in segs:
        upds[t0:t1] += sA[None, :, :]

    return upds, ca, errs
